# revision 6
# baseline (speedup 1.0000x reference)
"""3-layer GCN (DrugGCN) on 8 Trainium2 NeuronCores via Bass/Tile.

Strategy (node-sharded, dst-partitioned edges):
  - 50000 nodes split into 8 contiguous shards of 6250. Within each core the
    local node columns are padded so every graph's run starts at a multiple of
    8 (pooling windows), giving N_PAD columns per core.
  - Per layer: each core computes z = h @ W for its own nodes (TensorE,
    feature-major h in SBUF), writes z (fp16, node-major) to DRAM, AllGathers
    z across the 8 cores into a Shared DRAM tensor.
  - Edges are owned by the dst core, grouped by 512-wide dst block and src
    half (int16 gather index range). Edge messages are fetched with gpsimd
    dma_gather (one 256B row per edge) from the allgathered z. Scatter-add is
    a TensorE matmul per 128-edge tile against a segment matrix
    S[e, d] = norm_e * 1[dst_e == d] (d over the 512 block columns),
    accumulated in PSUM feature-major. S is generated on ScalarE (Abs+Relu
    delta trick) and VectorE (fused is_equal*mult) in a tunable ratio —
    ScalarE does not contend with the SWDGE descriptor generation that the
    gathers keep busy. Self loops are matmuls against a host-built diagonal
    deg_inv matrix. Epilogue relu(+bias) on ScalarE writes the next h.
  - Pooling: window sums/maxes over fixed 8-column windows (one VectorE
    reduce each); the host combines windows into per-graph mean/max.
"""
import numpy as np

import concourse.bacc as bacc
import concourse.mybir as mybir
import concourse.tile as tile
from concourse.bass_utils import run_bass_kernel_spmd
from concourse.library_config import mlp

NCORES = 8
N = 50000
E = 800000
G = 1600
F = 128
N_LOC = N // NCORES           # 6250
PAD_W = 8                     # pooling window width (columns)
BLK_W = 512                   # dst block width (one PSUM bank)
MAX_TILES_PER_GATHER = 32
ACT_SGEN_NUM, ACT_SGEN_DEN = 0, 5   # fraction of S tiles generated on ScalarE

_CACHE = {}


# ---------------------------------------------------------------- host prep

def _preprocess(edge_index, graph_index):
    src = np.asarray(edge_index[0], dtype=np.int64)
    dst = np.asarray(edge_index[1], dtype=np.int64)
    gi = np.asarray(graph_index, dtype=np.int64)

    deg = np.bincount(dst, minlength=N).astype(np.float64) + 1.0
    deg_isqrt = 1.0 / np.sqrt(deg)
    deg_inv = 1.0 / deg
    norm_e = (deg_isqrt[src] * deg_isqrt[dst]).astype(np.float32)

    # padded column layout per core: graph runs aligned to PAD_W
    col_of = np.zeros(N, dtype=np.int64)
    core_graphs = []
    npad_c = np.zeros(NCORES, dtype=np.int64)
    for c in range(NCORES):
        lo, hi = c * N_LOC, (c + 1) * N_LOC
        g_loc = gi[lo:hi]
        starts = np.flatnonzero(np.r_[True, g_loc[1:] != g_loc[:-1]])
        ends = np.r_[starts[1:], len(g_loc)]
        col = 0
        glist = []
        for s0, s1 in zip(starts, ends):
            col = -(-col // PAD_W) * PAD_W
            cnt = s1 - s0
            col_of[lo + s0:lo + s1] = col + np.arange(cnt)
            glist.append((int(g_loc[s0]), int(col), int(col + cnt)))
            col += cnt
        core_graphs.append(glist)
        npad_c[c] = col
    n_pad = int(-(-npad_c.max() // BLK_W) * BLK_W)
    assert 4 * n_pad < 32768, f"N_PAD={n_pad} too large for int16 gather idx"
    n_blk = n_pad // BLK_W                 # 512-wide blocks
    n_zt = n_pad // 128                    # 128-wide z tiles
    n_win = n_pad // PAD_W

    pad_gid = col_of + (np.arange(N) // N_LOC) * n_pad
    sec_of = pad_gid // (4 * n_pad)
    sec_idx = pad_gid - sec_of * (4 * n_pad)

    ecore = dst // N_LOC
    dcol = col_of[dst]
    dblk = dcol // BLK_W
    din = dcol % BLK_W

    esec = sec_of[src]
    order = np.lexsort((src, esec, dblk, ecore))
    e_sorted = order
    ec_s = ecore[order]
    blk_s = dblk[order]
    sec_s = esec[order]

    counts = np.zeros((NCORES, n_blk, 2), dtype=np.int64)
    np.add.at(counts, (ec_s, blk_s, sec_s), 1)
    cell_tiles = -(-counts.max(axis=0) // 128)          # [n_blk, 2]

    # table order: cell-major (block, then section), tiles consecutive
    tile_info = []          # (block, sec)
    cell_t0 = np.zeros((n_blk, 2), dtype=np.int64)
    for b in range(n_blk):
        for s in (0, 1):
            cell_t0[b, s] = len(tile_info)
            for _ in range(int(cell_tiles[b, s])):
                tile_info.append((b, s))
    t_total = len(tile_info)

    # per-core edge slot tables
    dstt = np.zeros((NCORES, 128, t_total), dtype=np.float32)
    normt = np.zeros((NCORES, 128, t_total), dtype=np.float32)
    idx_flat = np.zeros((NCORES, t_total * 128), dtype=np.int16)

    keys = (ec_s * n_blk + blk_s) * 2 + sec_s
    boundaries = np.flatnonzero(np.r_[True, keys[1:] != keys[:-1]])
    b_ends = np.r_[boundaries[1:], len(keys)]
    cell_start = {int(keys[bi]): (int(bi), int(be))
                  for bi, be in zip(boundaries, b_ends)}

    for c in range(NCORES):
        for b in range(n_blk):
            for s in (0, 1):
                key = (c * n_blk + b) * 2 + s
                if key not in cell_start:
                    continue
                i0, i1 = cell_start[key]
                edges = e_sorted[i0:i1]
                cnt = len(edges)
                si = sec_idx[src[edges]].astype(np.int16)
                dloc = din[edges].astype(np.float32)
                nv = norm_e[edges]
                t0 = int(cell_t0[b, s])
                p0 = t0 * 128
                idx_flat[c, p0:p0 + cnt] = si
                # slot (tile, partition) for edge k: tile t0 + k//128, part k%128
                for k0 in range(0, cnt, 128):
                    t = t0 + k0 // 128
                    n_here = min(128, cnt - k0)
                    dstt[c, :n_here, t] = dloc[k0:k0 + n_here]
                    normt[c, :n_here, t] = nv[k0:k0 + n_here]

    gidx = np.zeros((NCORES, 128, t_total * 8), dtype=np.int16)
    ar = np.arange(t_total * 128)
    for g in range(8):
        gidx[:, 16 * g + (ar % 16), ar // 16] = idx_flat

    dd = np.zeros((NCORES, 128, n_pad), dtype=np.float16)
    node_ids = np.arange(N)
    for c in range(NCORES):
        sel = node_ids[c * N_LOC:(c + 1) * N_LOC]
        cols = col_of[sel]
        p = cols % 128
        dd[c, p, cols] = deg_inv[sel].astype(np.float16)

    sched = dict(
        n_pad=n_pad, n_blk=n_blk, n_zt=n_zt, n_win=n_win, t_total=t_total,
        cell_tiles=cell_tiles, cell_t0=cell_t0,
        core_graphs=core_graphs, col_of=col_of,
    )
    tables = dict(gidx=gidx, dstt=dstt, normt=normt, dd=dd)
    return sched, tables


# ---------------------------------------------------------------- program

def _build_program(sched):
    n_pad = sched["n_pad"]
    n_blk = sched["n_blk"]
    n_zt = sched["n_zt"]
    n_win = sched["n_win"]
    t_total = sched["t_total"]
    cell_tiles = sched["cell_tiles"]
    cell_t0 = sched["cell_t0"]

    f16, f32, i16 = mybir.dt.float16, mybir.dt.float32, mybir.dt.int16

    nc = bacc.Bacc("TRN2", target_bir_lowering=False, debug=False,
                   num_devices=NCORES)

    xT_in = nc.dram_tensor("xT", [128, n_pad], f16, kind="ExternalInput")
    gidx_in = nc.dram_tensor("gidx", [128, t_total * 8], i16, kind="ExternalInput")
    dstt_in = nc.dram_tensor("dstt", [128, t_total], f32, kind="ExternalInput")
    normt_in = nc.dram_tensor("normt", [128, t_total], f32, kind="ExternalInput")
    ndstt_in = nc.dram_tensor("ndstt", [128, t_total], f32, kind="ExternalInput")
    nnormt_in = nc.dram_tensor("nnormt", [128, t_total], f32, kind="ExternalInput")
    dd_in = nc.dram_tensor("dd", [128, n_pad], f16, kind="ExternalInput")
    iota_in = nc.dram_tensor("iota", [128, BLK_W], f32, kind="ExternalInput")
    W_in = [nc.dram_tensor(f"W{i}", [128, 128], f16, kind="ExternalInput")
            for i in range(3)]
    b_in = [nc.dram_tensor(f"b{i}", [128, 1], f32, kind="ExternalInput")
            for i in range(3)]
    wsum_out = nc.dram_tensor("wsums", [128, n_win], f32, kind="ExternalOutput")
    wmax_out = nc.dram_tensor("wmaxs", [128, n_win], f32, kind="ExternalOutput")

    z_loc = [nc.dram_tensor(f"z_loc{i}", [n_pad, 128], f16) for i in range(3)]
    z_full = [nc.dram_tensor(f"z_full{i}", [NCORES * n_pad, 128], f16,
                             addr_space="Shared") for i in range(3)]

    with tile.TileContext(nc) as tc:
        with (
            tc.tile_pool(name="const", bufs=1) as constp,
            tc.tile_pool(name="hbuf", bufs=2) as hpool,
            tc.tile_pool(name="zbuf", bufs=2) as zpool,
            tc.tile_pool(name="msg", bufs=4) as msgpool,
            tc.tile_pool(name="spool", bufs=8) as spool,
            tc.tile_pool(name="apool", bufs=4) as apool,
            tc.tile_pool(name="zps", bufs=2, space="PSUM") as zpsum,
            tc.tile_pool(name="aggps", bufs=3, space="PSUM") as aggpsum,
            tc.tile_pool(name="outp", bufs=1) as outp,
        ):
            nc.gpsimd.load_library(mlp)

            gidx_sb = constp.tile([128, t_total * 8], i16, tag="gidx")
            nc.sync.dma_start(gidx_sb[:], gidx_in[:])
            dstt_sb = constp.tile([128, t_total], f32, tag="dstt")
            nc.sync.dma_start(dstt_sb[:], dstt_in[:])
            normt_sb = constp.tile([128, t_total], f32, tag="normt")
            nc.sync.dma_start(normt_sb[:], normt_in[:])
            ndstt_sb = constp.tile([128, t_total], f32, tag="ndstt")
            nc.sync.dma_start(ndstt_sb[:], ndstt_in[:])
            nnormt_sb = constp.tile([128, t_total], f32, tag="nnormt")
            nc.sync.dma_start(nnormt_sb[:], nnormt_in[:])
            dd_sb = constp.tile([128, n_pad], f16, tag="dd")
            nc.sync.dma_start(dd_sb[:], dd_in[:])
            iota_sb = constp.tile([128, BLK_W], f32, tag="iota")
            nc.sync.dma_start(iota_sb[:], iota_in[:])
            W_sb = []
            b_sb = []
            for i in range(3):
                w = constp.tile([128, 128], f16, tag=f"W{i}")
                nc.sync.dma_start(w[:], W_in[i][:])
                W_sb.append(w)
                b = constp.tile([128, 1], f32, tag=f"b{i}")
                nc.sync.dma_start(b[:], b_in[i][:])
                b_sb.append(b)

            h_cur = hpool.tile([128, n_pad], f16, tag="h")
            nc.sync.dma_start(h_cur[:], xT_in[:])

            zrow = constp.tile([1, BLK_W], f16, tag="zrow")
            nc.vector.memset(zrow[:], 0.0)
            zcol = constp.tile([1, 128], f16, tag="zcol")
            nc.vector.memset(zcol[:], 0.0)

            relu = mybir.ActivationFunctionType.Relu
            absf = mybir.ActivationFunctionType.Abs
            sgen_ctr = 0

            for lay in range(3):
                # ---- z = h @ W (node-major 128-wide tiles), stage to DRAM
                z_sb = zpool.tile([128, n_zt, 128], f16, tag="zsb")
                for j in range(n_zt):
                    z_ps = zpsum.tile([128, 128], f32, tag="zps")
                    nc.tensor.matmul(z_ps[:], h_cur[:, j * 128:(j + 1) * 128],
                                     W_sb[lay][:], start=True, stop=True)
                    nc.vector.tensor_copy(z_sb[:, j, :], z_ps[:])
                    nc.sync.dma_start(z_loc[lay][j * 128:(j + 1) * 128, :],
                                      z_sb[:, j, :])
                nc.gpsimd.collective_compute(
                    "AllGather", mybir.AluOpType.bypass,
                    replica_groups=[list(range(NCORES))],
                    ins=[z_loc[lay][:]], outs=[z_full[lay][:]],
                )
                zsec = (z_full[lay][0:4 * n_pad, :],
                        z_full[lay][4 * n_pad:8 * n_pad, :])

                h_next = hpool.tile([128, n_pad], f16, tag="h")

                # ---- edge aggregation per 512-wide block
                for blk in range(n_blk):
                    n_tiles = int(cell_tiles[blk, 0] + cell_tiles[blk, 1])
                    agg = aggpsum.tile([128, BLK_W], f32, tag="agg")
                    # zero-fill the whole bank (clears has_written per element)
                    nc.tensor.matmul(agg[:], zcol[:], zrow[:],
                                     start=True, stop=False)
                    for k in range(BLK_W // 128):
                        j = blk * (BLK_W // 128) + k
                        nc.tensor.matmul(
                            agg[:, k * 128:(k + 1) * 128],
                            z_sb[:, j, :], dd_sb[:, j * 128:(j + 1) * 128],
                            start=False,
                            stop=(n_tiles == 0 and k == BLK_W // 128 - 1))
                    done = 0
                    for s in (0, 1):
                        nt = int(cell_tiles[blk, s])
                        t0 = int(cell_t0[blk, s])
                        for c0 in range(0, nt, MAX_TILES_PER_GATHER):
                            cn = min(MAX_TILES_PER_GATHER, nt - c0)
                            msg = msgpool.tile([128, MAX_TILES_PER_GATHER, 128],
                                               f16, tag="msg")
                            g0 = t0 + c0
                            nc.gpsimd.dma_gather(
                                msg[:, 0:cn, :], zsec[s],
                                gidx_sb[:, g0 * 8:(g0 + cn) * 8],
                                cn * 128, cn * 128, 128, single_packet=False)
                            for k in range(cn):
                                tcol = g0 + k
                                s_t = spool.tile([128, BLK_W], f16, tag="S")
                                if (sgen_ctr % ACT_SGEN_DEN) < ACT_SGEN_NUM:
                                    a_t = apool.tile([128, BLK_W], f32, tag="A")
                                    nc.scalar.activation(
                                        a_t[:], iota_sb[:], absf,
                                        bias=ndstt_sb[:, tcol:tcol + 1],
                                        scale=1.0)
                                    nc.scalar.activation(
                                        s_t[:], a_t[:], relu,
                                        bias=normt_sb[:, tcol:tcol + 1],
                                        scale=nnormt_sb[:, tcol:tcol + 1])
                                else:
                                    nc.vector.tensor_scalar(
                                        s_t[:], iota_sb[:],
                                        dstt_sb[:, tcol:tcol + 1],
                                        normt_sb[:, tcol:tcol + 1],
                                        mybir.AluOpType.is_equal,
                                        mybir.AluOpType.mult)
                                sgen_ctr += 1
                                done += 1
                                nc.tensor.matmul(agg[:], msg[:, k, :], s_t[:],
                                                 start=False,
                                                 stop=(done == n_tiles))
                    nc.scalar.activation(
                        h_next[:, blk * BLK_W:(blk + 1) * BLK_W], agg[:],
                        relu, bias=b_sb[lay][:])
                h_cur = h_next

            # ---- pooling: window sums / maxes
            ws_sb = outp.tile([128, n_win], f32, tag="ws")
            wm_sb = outp.tile([128, n_win], f32, tag="wm")
            h3 = h_cur[:].rearrange("p (w k) -> p w k", k=PAD_W)
            nc.vector.tensor_reduce(ws_sb[:], h3, mybir.AxisListType.X,
                                    mybir.AluOpType.add)
            nc.vector.tensor_reduce(wm_sb[:], h3, mybir.AxisListType.X,
                                    mybir.AluOpType.max)
            nc.sync.dma_start(wsum_out[:], ws_sb[:])
            nc.sync.dma_start(wmax_out[:], wm_sb[:])

    nc.compile()
    return nc


# ---------------------------------------------------------------- kernel

def make_in_maps(inputs, sched, tables):
    n_pad = sched["n_pad"]
    col_of = sched["col_of"]
    x = np.asarray(inputs["x"], dtype=np.float32)
    Ws = [np.asarray(inputs[k], dtype=np.float32) for k in ("W1", "W2", "W3")]
    bs = [np.asarray(inputs[k], dtype=np.float32) for k in ("b1", "b2", "b3")]
    iota = np.tile(np.arange(BLK_W, dtype=np.float32), (128, 1))
    in_maps = []
    for c in range(NCORES):
        sel = np.arange(c * N_LOC, (c + 1) * N_LOC)
        xT = np.zeros((128, n_pad), dtype=np.float16)
        xT[:, col_of[sel]] = x[sel].T.astype(np.float16)
        m = {
            "xT": xT,
            "gidx": tables["gidx"][c],
            "dstt": tables["dstt"][c],
            "normt": tables["normt"][c],
            "ndstt": -tables["dstt"][c],
            "nnormt": -tables["normt"][c],
            "dd": tables["dd"][c],
            "iota": iota,
        }
        for i in range(3):
            m[f"W{i}"] = Ws[i].astype(np.float16)
            m[f"b{i}"] = bs[i].reshape(128, 1)
        in_maps.append(m)
    return in_maps


def kernel(x, edge_index, graph_index, W1, b1, W2, b2, W3, b3):
    key = "gcn"
    if key not in _CACHE:
        sched, tables = _preprocess(edge_index, graph_index)
        nc = _build_program(sched)
        _CACHE[key] = (sched, tables, nc)
    sched, tables, nc = _CACHE[key]

    inputs = dict(x=x, W1=W1, b1=b1, W2=W2, b2=b2, W3=W3, b3=b3)
    in_maps = make_in_maps(inputs, sched, tables)
    res = run_bass_kernel_spmd(nc, in_maps, list(range(NCORES)))
    return _combine(res.results, sched, graph_index)


def _combine(results, sched, graph_index):
    gi = np.asarray(graph_index, dtype=np.int64)
    counts = np.bincount(gi, minlength=G).astype(np.float64)
    sums = np.zeros((G, F), dtype=np.float64)
    maxs = np.full((G, F), -np.inf, dtype=np.float64)
    for c in range(NCORES):
        ws = results[c]["wsums"].astype(np.float64)
        wm = results[c]["wmaxs"]
        for (g, c0, c1) in sched["core_graphs"][c]:
            w0, w1 = c0 // PAD_W, -(-c1 // PAD_W)
            sums[g] += ws[:, w0:w1].sum(axis=1)
            maxs[g] = np.maximum(maxs[g], wm[:, w0:w1].max(axis=1))
    mean = sums / np.maximum(counts, 1.0)[:, None]
    out = np.concatenate([mean, maxs], axis=-1).astype(np.float32)
    return out


# revision 7
# speedup vs baseline: 1.1004x; 1.1004x over previous
"""3-layer GCN (DrugGCN) on 8 Trainium2 NeuronCores via Bass/Tile.

Strategy (node-sharded, dst-partitioned edges):
  - 50000 nodes split into 8 contiguous shards of 6250. Within each core the
    local node columns are padded so every graph's run starts at a multiple of
    8 (pooling windows), giving N_PAD columns per core.
  - Per layer: each core computes z = h @ W for its own nodes (TensorE,
    feature-major h in SBUF), writes z (fp16, node-major) to DRAM, AllGathers
    z across the 8 cores into a Shared DRAM tensor.
  - Edges are owned by the dst core, grouped by 512-wide dst block and src
    half (int16 gather index range). Edge messages are fetched with gpsimd
    dma_gather (one 256B row per edge) from the allgathered z. Scatter-add is
    a TensorE matmul per 128-edge tile against a segment matrix
    S[e, d] = norm_e * 1[dst_e == d] (d over the 512 block columns),
    accumulated in PSUM feature-major. S is generated on ScalarE (Abs+Relu
    delta trick) and VectorE (fused is_equal*mult) in a tunable ratio —
    ScalarE does not contend with the SWDGE descriptor generation that the
    gathers keep busy. Self loops are matmuls against a host-built diagonal
    deg_inv matrix. Epilogue relu(+bias) on ScalarE writes the next h.
  - Pooling: window sums/maxes over fixed 8-column windows (one VectorE
    reduce each); the host combines windows into per-graph mean/max.
"""
import numpy as np

import concourse.bacc as bacc
import concourse.mybir as mybir
import concourse.tile as tile
from concourse.bass_utils import run_bass_kernel_spmd
from concourse.library_config import mlp

NCORES = 8
N = 50000
E = 800000
G = 1600
F = 128
N_LOC = N // NCORES           # 6250
PAD_W = 8                     # pooling window width (columns)
BLK_W = 512                   # dst block width (one PSUM bank)
MAX_TILES_PER_GATHER = 32
ACT_SGEN_NUM, ACT_SGEN_DEN = 3, 5   # fraction of S tiles generated on ScalarE

_CACHE = {}


# ---------------------------------------------------------------- host prep

def _preprocess(edge_index, graph_index):
    src = np.asarray(edge_index[0], dtype=np.int64)
    dst = np.asarray(edge_index[1], dtype=np.int64)
    gi = np.asarray(graph_index, dtype=np.int64)

    deg = np.bincount(dst, minlength=N).astype(np.float64) + 1.0
    deg_isqrt = 1.0 / np.sqrt(deg)
    deg_inv = 1.0 / deg
    norm_e = (deg_isqrt[src] * deg_isqrt[dst]).astype(np.float32)

    # padded column layout per core: graph runs aligned to PAD_W
    col_of = np.zeros(N, dtype=np.int64)
    core_graphs = []
    npad_c = np.zeros(NCORES, dtype=np.int64)
    for c in range(NCORES):
        lo, hi = c * N_LOC, (c + 1) * N_LOC
        g_loc = gi[lo:hi]
        starts = np.flatnonzero(np.r_[True, g_loc[1:] != g_loc[:-1]])
        ends = np.r_[starts[1:], len(g_loc)]
        col = 0
        glist = []
        for s0, s1 in zip(starts, ends):
            col = -(-col // PAD_W) * PAD_W
            cnt = s1 - s0
            col_of[lo + s0:lo + s1] = col + np.arange(cnt)
            glist.append((int(g_loc[s0]), int(col), int(col + cnt)))
            col += cnt
        core_graphs.append(glist)
        npad_c[c] = col
    n_pad = int(-(-npad_c.max() // BLK_W) * BLK_W)
    assert 4 * n_pad < 32768, f"N_PAD={n_pad} too large for int16 gather idx"
    n_blk = n_pad // BLK_W                 # 512-wide blocks
    n_zt = n_pad // 128                    # 128-wide z tiles
    n_win = n_pad // PAD_W

    pad_gid = col_of + (np.arange(N) // N_LOC) * n_pad
    sec_of = pad_gid // (4 * n_pad)
    sec_idx = pad_gid - sec_of * (4 * n_pad)

    ecore = dst // N_LOC
    dcol = col_of[dst]
    dblk = dcol // BLK_W
    din = dcol % BLK_W

    esec = sec_of[src]
    order = np.lexsort((src, esec, dblk, ecore))
    e_sorted = order
    ec_s = ecore[order]
    blk_s = dblk[order]
    sec_s = esec[order]

    counts = np.zeros((NCORES, n_blk, 2), dtype=np.int64)
    np.add.at(counts, (ec_s, blk_s, sec_s), 1)
    cell_tiles = -(-counts.max(axis=0) // 128)          # [n_blk, 2]

    # table order: cell-major (block, then section), tiles consecutive
    tile_info = []          # (block, sec)
    cell_t0 = np.zeros((n_blk, 2), dtype=np.int64)
    for b in range(n_blk):
        for s in (0, 1):
            cell_t0[b, s] = len(tile_info)
            for _ in range(int(cell_tiles[b, s])):
                tile_info.append((b, s))
    t_total = len(tile_info)

    # per-core edge slot tables
    dstt = np.zeros((NCORES, 128, t_total), dtype=np.float32)
    normt = np.zeros((NCORES, 128, t_total), dtype=np.float32)
    idx_flat = np.zeros((NCORES, t_total * 128), dtype=np.int16)

    keys = (ec_s * n_blk + blk_s) * 2 + sec_s
    boundaries = np.flatnonzero(np.r_[True, keys[1:] != keys[:-1]])
    b_ends = np.r_[boundaries[1:], len(keys)]
    cell_start = {int(keys[bi]): (int(bi), int(be))
                  for bi, be in zip(boundaries, b_ends)}

    for c in range(NCORES):
        for b in range(n_blk):
            for s in (0, 1):
                key = (c * n_blk + b) * 2 + s
                if key not in cell_start:
                    continue
                i0, i1 = cell_start[key]
                edges = e_sorted[i0:i1]
                cnt = len(edges)
                si = sec_idx[src[edges]].astype(np.int16)
                dloc = din[edges].astype(np.float32)
                nv = norm_e[edges]
                t0 = int(cell_t0[b, s])
                p0 = t0 * 128
                idx_flat[c, p0:p0 + cnt] = si
                # slot (tile, partition) for edge k: tile t0 + k//128, part k%128
                for k0 in range(0, cnt, 128):
                    t = t0 + k0 // 128
                    n_here = min(128, cnt - k0)
                    dstt[c, :n_here, t] = dloc[k0:k0 + n_here]
                    normt[c, :n_here, t] = nv[k0:k0 + n_here]

    gidx = np.zeros((NCORES, 128, t_total * 8), dtype=np.int16)
    ar = np.arange(t_total * 128)
    for g in range(8):
        gidx[:, 16 * g + (ar % 16), ar // 16] = idx_flat

    dd = np.zeros((NCORES, 128, n_pad), dtype=np.float16)
    node_ids = np.arange(N)
    for c in range(NCORES):
        sel = node_ids[c * N_LOC:(c + 1) * N_LOC]
        cols = col_of[sel]
        p = cols % 128
        dd[c, p, cols] = deg_inv[sel].astype(np.float16)

    sched = dict(
        n_pad=n_pad, n_blk=n_blk, n_zt=n_zt, n_win=n_win, t_total=t_total,
        cell_tiles=cell_tiles, cell_t0=cell_t0,
        core_graphs=core_graphs, col_of=col_of,
    )
    tables = dict(gidx=gidx, dstt=dstt, normt=normt, dd=dd)
    return sched, tables


# ---------------------------------------------------------------- program

def _build_program(sched):
    n_pad = sched["n_pad"]
    n_blk = sched["n_blk"]
    n_zt = sched["n_zt"]
    n_win = sched["n_win"]
    t_total = sched["t_total"]
    cell_tiles = sched["cell_tiles"]
    cell_t0 = sched["cell_t0"]

    f16, f32, i16 = mybir.dt.float16, mybir.dt.float32, mybir.dt.int16

    nc = bacc.Bacc("TRN2", target_bir_lowering=False, debug=False,
                   num_devices=NCORES)

    xT_in = nc.dram_tensor("xT", [128, n_pad], f16, kind="ExternalInput")
    gidx_in = nc.dram_tensor("gidx", [128, t_total * 8], i16, kind="ExternalInput")
    dstt_in = nc.dram_tensor("dstt", [128, t_total], f32, kind="ExternalInput")
    normt_in = nc.dram_tensor("normt", [128, t_total], f32, kind="ExternalInput")
    ndstt_in = nc.dram_tensor("ndstt", [128, t_total], f32, kind="ExternalInput")
    nnormt_in = nc.dram_tensor("nnormt", [128, t_total], f32, kind="ExternalInput")
    dd_in = nc.dram_tensor("dd", [128, n_pad], f16, kind="ExternalInput")
    iota_in = nc.dram_tensor("iota", [128, BLK_W], f32, kind="ExternalInput")
    W_in = [nc.dram_tensor(f"W{i}", [128, 128], f16, kind="ExternalInput")
            for i in range(3)]
    b_in = [nc.dram_tensor(f"b{i}", [128, 1], f32, kind="ExternalInput")
            for i in range(3)]
    wsum_out = nc.dram_tensor("wsums", [128, n_win], f32, kind="ExternalOutput")
    wmax_out = nc.dram_tensor("wmaxs", [128, n_win], f32, kind="ExternalOutput")

    z_loc = [nc.dram_tensor(f"z_loc{i}", [n_pad, 128], f16) for i in range(3)]
    z_full = [nc.dram_tensor(f"z_full{i}", [NCORES * n_pad, 128], f16,
                             addr_space="Shared") for i in range(3)]

    with tile.TileContext(nc) as tc:
        with (
            tc.tile_pool(name="const", bufs=1) as constp,
            tc.tile_pool(name="hbuf", bufs=2) as hpool,
            tc.tile_pool(name="zbuf", bufs=2) as zpool,
            tc.tile_pool(name="msg", bufs=4) as msgpool,
            tc.tile_pool(name="spool", bufs=8) as spool,
            tc.tile_pool(name="apool", bufs=4) as apool,
            tc.tile_pool(name="zps", bufs=2, space="PSUM") as zpsum,
            tc.tile_pool(name="aggps", bufs=3, space="PSUM") as aggpsum,
            tc.tile_pool(name="outp", bufs=1) as outp,
        ):
            nc.gpsimd.load_library(mlp)

            gidx_sb = constp.tile([128, t_total * 8], i16, tag="gidx")
            nc.sync.dma_start(gidx_sb[:], gidx_in[:])
            dstt_sb = constp.tile([128, t_total], f32, tag="dstt")
            nc.sync.dma_start(dstt_sb[:], dstt_in[:])
            normt_sb = constp.tile([128, t_total], f32, tag="normt")
            nc.sync.dma_start(normt_sb[:], normt_in[:])
            ndstt_sb = constp.tile([128, t_total], f32, tag="ndstt")
            nc.sync.dma_start(ndstt_sb[:], ndstt_in[:])
            nnormt_sb = constp.tile([128, t_total], f32, tag="nnormt")
            nc.sync.dma_start(nnormt_sb[:], nnormt_in[:])
            dd_sb = constp.tile([128, n_pad], f16, tag="dd")
            nc.sync.dma_start(dd_sb[:], dd_in[:])
            iota_sb = constp.tile([128, BLK_W], f32, tag="iota")
            nc.sync.dma_start(iota_sb[:], iota_in[:])
            W_sb = []
            b_sb = []
            for i in range(3):
                w = constp.tile([128, 128], f16, tag=f"W{i}")
                nc.sync.dma_start(w[:], W_in[i][:])
                W_sb.append(w)
                b = constp.tile([128, 1], f32, tag=f"b{i}")
                nc.sync.dma_start(b[:], b_in[i][:])
                b_sb.append(b)

            h_cur = hpool.tile([128, n_pad], f16, tag="h")
            nc.sync.dma_start(h_cur[:], xT_in[:])

            zrow = constp.tile([1, BLK_W], f16, tag="zrow")
            nc.vector.memset(zrow[:], 0.0)
            zcol = constp.tile([1, 128], f16, tag="zcol")
            nc.vector.memset(zcol[:], 0.0)

            relu = mybir.ActivationFunctionType.Relu
            absf = mybir.ActivationFunctionType.Abs
            sgen_ctr = 0

            for lay in range(3):
                # ---- z = h @ W (node-major 128-wide tiles), stage to DRAM
                z_sb = zpool.tile([128, n_zt, 128], f16, tag="zsb")
                for j in range(n_zt):
                    z_ps = zpsum.tile([128, 128], f32, tag="zps")
                    nc.tensor.matmul(z_ps[:], h_cur[:, j * 128:(j + 1) * 128],
                                     W_sb[lay][:], start=True, stop=True)
                    nc.vector.tensor_copy(z_sb[:, j, :], z_ps[:])
                    nc.sync.dma_start(z_loc[lay][j * 128:(j + 1) * 128, :],
                                      z_sb[:, j, :])
                nc.gpsimd.collective_compute(
                    "AllGather", mybir.AluOpType.bypass,
                    replica_groups=[list(range(NCORES))],
                    ins=[z_loc[lay][:]], outs=[z_full[lay][:]],
                )
                zsec = (z_full[lay][0:4 * n_pad, :],
                        z_full[lay][4 * n_pad:8 * n_pad, :])

                h_next = hpool.tile([128, n_pad], f16, tag="h")

                # ---- edge aggregation per 512-wide block
                for blk in range(n_blk):
                    n_tiles = int(cell_tiles[blk, 0] + cell_tiles[blk, 1])
                    agg = aggpsum.tile([128, BLK_W], f32, tag="agg")
                    # zero-fill the whole bank (clears has_written per element)
                    nc.tensor.matmul(agg[:], zcol[:], zrow[:],
                                     start=True, stop=False)
                    for k in range(BLK_W // 128):
                        j = blk * (BLK_W // 128) + k
                        nc.tensor.matmul(
                            agg[:, k * 128:(k + 1) * 128],
                            z_sb[:, j, :], dd_sb[:, j * 128:(j + 1) * 128],
                            start=False,
                            stop=(n_tiles == 0 and k == BLK_W // 128 - 1))
                    done = 0
                    for s in (0, 1):
                        nt = int(cell_tiles[blk, s])
                        t0 = int(cell_t0[blk, s])
                        for c0 in range(0, nt, MAX_TILES_PER_GATHER):
                            cn = min(MAX_TILES_PER_GATHER, nt - c0)
                            msg = msgpool.tile([128, MAX_TILES_PER_GATHER, 128],
                                               f16, tag="msg")
                            g0 = t0 + c0
                            nc.gpsimd.dma_gather(
                                msg[:, 0:cn, :], zsec[s],
                                gidx_sb[:, g0 * 8:(g0 + cn) * 8],
                                cn * 128, cn * 128, 128, single_packet=False)
                            for k in range(cn):
                                tcol = g0 + k
                                s_t = spool.tile([128, BLK_W], f16, tag="S")
                                if (sgen_ctr % ACT_SGEN_DEN) < ACT_SGEN_NUM:
                                    a_t = apool.tile([128, BLK_W], f32, tag="A")
                                    nc.scalar.activation(
                                        a_t[:], iota_sb[:], absf,
                                        bias=ndstt_sb[:, tcol:tcol + 1],
                                        scale=1.0)
                                    nc.scalar.activation(
                                        s_t[:], a_t[:], relu,
                                        bias=normt_sb[:, tcol:tcol + 1],
                                        scale=nnormt_sb[:, tcol:tcol + 1])
                                else:
                                    nc.vector.tensor_scalar(
                                        s_t[:], iota_sb[:],
                                        dstt_sb[:, tcol:tcol + 1],
                                        normt_sb[:, tcol:tcol + 1],
                                        mybir.AluOpType.is_equal,
                                        mybir.AluOpType.mult)
                                sgen_ctr += 1
                                done += 1
                                nc.tensor.matmul(agg[:], msg[:, k, :], s_t[:],
                                                 start=False,
                                                 stop=(done == n_tiles))
                    nc.scalar.activation(
                        h_next[:, blk * BLK_W:(blk + 1) * BLK_W], agg[:],
                        relu, bias=b_sb[lay][:])
                h_cur = h_next

            # ---- pooling: window sums / maxes
            ws_sb = outp.tile([128, n_win], f32, tag="ws")
            wm_sb = outp.tile([128, n_win], f32, tag="wm")
            h3 = h_cur[:].rearrange("p (w k) -> p w k", k=PAD_W)
            nc.vector.tensor_reduce(ws_sb[:], h3, mybir.AxisListType.X,
                                    mybir.AluOpType.add)
            nc.vector.tensor_reduce(wm_sb[:], h3, mybir.AxisListType.X,
                                    mybir.AluOpType.max)
            nc.sync.dma_start(wsum_out[:], ws_sb[:])
            nc.sync.dma_start(wmax_out[:], wm_sb[:])

    nc.compile()
    return nc


# ---------------------------------------------------------------- kernel

def make_in_maps(inputs, sched, tables):
    n_pad = sched["n_pad"]
    col_of = sched["col_of"]
    x = np.asarray(inputs["x"], dtype=np.float32)
    Ws = [np.asarray(inputs[k], dtype=np.float32) for k in ("W1", "W2", "W3")]
    bs = [np.asarray(inputs[k], dtype=np.float32) for k in ("b1", "b2", "b3")]
    iota = np.tile(np.arange(BLK_W, dtype=np.float32), (128, 1))
    in_maps = []
    for c in range(NCORES):
        sel = np.arange(c * N_LOC, (c + 1) * N_LOC)
        xT = np.zeros((128, n_pad), dtype=np.float16)
        xT[:, col_of[sel]] = x[sel].T.astype(np.float16)
        m = {
            "xT": xT,
            "gidx": tables["gidx"][c],
            "dstt": tables["dstt"][c],
            "normt": tables["normt"][c],
            "ndstt": -tables["dstt"][c],
            "nnormt": -tables["normt"][c],
            "dd": tables["dd"][c],
            "iota": iota,
        }
        for i in range(3):
            m[f"W{i}"] = Ws[i].astype(np.float16)
            m[f"b{i}"] = bs[i].reshape(128, 1)
        in_maps.append(m)
    return in_maps


def kernel(x, edge_index, graph_index, W1, b1, W2, b2, W3, b3):
    key = "gcn"
    if key not in _CACHE:
        sched, tables = _preprocess(edge_index, graph_index)
        nc = _build_program(sched)
        _CACHE[key] = (sched, tables, nc)
    sched, tables, nc = _CACHE[key]

    inputs = dict(x=x, W1=W1, b1=b1, W2=W2, b2=b2, W3=W3, b3=b3)
    in_maps = make_in_maps(inputs, sched, tables)
    res = run_bass_kernel_spmd(nc, in_maps, list(range(NCORES)))
    return _combine(res.results, sched, graph_index)


def _combine(results, sched, graph_index):
    gi = np.asarray(graph_index, dtype=np.int64)
    counts = np.bincount(gi, minlength=G).astype(np.float64)
    sums = np.zeros((G, F), dtype=np.float64)
    maxs = np.full((G, F), -np.inf, dtype=np.float64)
    for c in range(NCORES):
        ws = results[c]["wsums"].astype(np.float64)
        wm = results[c]["wmaxs"]
        for (g, c0, c1) in sched["core_graphs"][c]:
            w0, w1 = c0 // PAD_W, -(-c1 // PAD_W)
            sums[g] += ws[:, w0:w1].sum(axis=1)
            maxs[g] = np.maximum(maxs[g], wm[:, w0:w1].max(axis=1))
    mean = sums / np.maximum(counts, 1.0)[:, None]
    out = np.concatenate([mean, maxs], axis=-1).astype(np.float32)
    return out
